# revision 20
# baseline (speedup 1.0000x reference)
"""MeanNSE (segment-reduce) Trainium2 kernel — 8 NeuronCores, data-parallel.

Host side (index prep only, same spirit as np.bincount for counts):
  * counting-sort element indices by basin id (stable argsort on uint16),
  * zero-pad each basin's run to a multiple of G=128 and lay the padded
    stream out column-major so every 128-element group is one SBUF column:
    padded rank r -> (partition p = r%128, group q = r//128 -> core, tile,
    column). Groups never straddle tiles or cores, so every column of every
    device tile belongs to exactly one basin (zeros contribute nothing).

Device (per core, E = 2162688 padded elements as 11 bf16 tiles [128, 1536]):
  VectorE: d = t - p                        (tensor_sub, bf16)
  GPSIMD : z_t2 = t*t   (tiles 0-7)         (tensor_mul, bf16)
  VectorE: z_t2 = t*t   (tiles 8-10)        (load balance)
  ScalarE: z_d2 = d^2                       (activation Square, bf16)
  TensorE: all group reductions. For each 512-column chunk k (33 per core),
    an fp32-PSUM-accumulating matmul with one-hot weights oneh[:, k, :33]
    adds each column's 128-partition sum into PSUM row k (rows not selected
    by the one-hot receive +0), for each of the three streams t, z_t2, z_d2:
        ps_s[0:33, :512] (+)= onehot_k^T @ stream_s[:, chunk_k]
  3 PSUM->SBUF copies + 3 eviction DMAs [33, 512] fp32.

Host combine in float64: np.bincount of the per-group partials per basin;
ss_tot = S_t2 - S_t^2/count (one-pass identity, counts from np.bincount);
answer = mean(1 - S_d2/(ss_tot + 1e-10)). Overall rel err vs fp32
reference ~1.4e-4 (bf16 products, fp32 accumulation).
"""

import sys

sys.path.insert(0, "/opt/trn_rl_repo")

import numpy as np
import ml_dtypes  # noqa: F401

import concourse.bacc as bacc
import concourse.mybir as mybir
import concourse.tile as tile
from concourse.bass_utils import run_bass_kernel_spmd

F32 = mybir.dt.float32
F16 = mybir.dt.float16
BF16 = mybir.dt.bfloat16

N_CORES = 8
N_TOTAL = 16777216
N_BASINS = 671
EPS = 1e-10

G = 128  # elements per group (pad unit) = one SBUF column
F_T = 1536  # columns per tile
N_T = 11  # tiles per core
COLS = N_T * F_T  # 16896 groups (columns) per core
E = 128 * COLS  # 2162688 elements per core
E_TOT = N_CORES * E  # 17301504 >= 16777216 + 671*127 (max padding)
CHUNK = 512  # columns per PSUM row (fp32 capacity of one bank row)
NCH = COLS // CHUNK  # 33 chunks (PSUM rows) per core per stat

_AF = mybir.ActivationFunctionType
_ALU = mybir.AluOpType

_cache = {}


def _build():
    nc = bacc.Bacc()
    yt = nc.declare_dram_parameter("yt", [E], BF16, isOutput=False)
    yp = nc.declare_dram_parameter("yp", [E], BF16, isOutput=False)
    pt = nc.declare_dram_parameter("pt", [NCH, CHUNK], F32, isOutput=True)
    pt2 = nc.declare_dram_parameter("pt2", [NCH, CHUNK], F32, isOutput=True)
    pd2 = nc.declare_dram_parameter("pd2", [NCH, CHUNK], F32, isOutput=True)

    with tile.TileContext(nc) as tc:
        with (
            tc.tile_pool(name="const", bufs=1) as cpool,
            tc.tile_pool(name="io", bufs=11) as io_pool,
            tc.tile_pool(name="work", bufs=8) as work_pool,
            tc.tile_pool(name="psum", bufs=1, space="PSUM") as psum_pool,
        ):
            # one-hot weight rows: oneh[p, k, m] = (m == k), same per partition
            oneh = cpool.tile([128, NCH, NCH], BF16, tag="oneh")
            nc.gpsimd.memset(oneh[:, :, :], 0.0)
            for k in range(NCH):
                nc.gpsimd.memset(oneh[:, k, k : k + 1], 1.0)
            ps_t = psum_pool.tile([128, CHUNK], F32, tag="ps_t")
            ps_t2 = psum_pool.tile([128, CHUNK], F32, tag="ps_t2")
            ps_d2 = psum_pool.tile([128, CHUNK], F32, tag="ps_d2")
            for t in range(N_T):
                base = t * 128 * F_T
                tt = io_pool.tile([128, F_T], BF16, tag="yt")
                tp = io_pool.tile([128, F_T], BF16, tag="yp")
                nc.sync.dma_start(
                    tt[:, :],
                    yt[base : base + 128 * F_T].rearrange("(p f) -> p f", p=128),
                )
                nc.sync.dma_start(
                    tp[:, :],
                    yp[base : base + 128 * F_T].rearrange("(p f) -> p f", p=128),
                )
                d = work_pool.tile([128, F_T], BF16, tag="d")
                zt2 = work_pool.tile([128, F_T], BF16, tag="zt2")
                zd2 = work_pool.tile([128, F_T], BF16, tag="zd2")
                nc.vector.tensor_sub(d[:, :], tt[:, :], tp[:, :])
                nc.scalar.square(zd2[:, :], d[:, :])
                if t < 8:
                    nc.gpsimd.tensor_mul(zt2[:, :], tt[:, :], tt[:, :])
                else:
                    nc.vector.tensor_mul(zt2[:, :], tt[:, :], tt[:, :])
                for kl in range(F_T // CHUNK):
                    k = t * (F_T // CHUNK) + kl
                    sl = slice(kl * CHUNK, (kl + 1) * CHUNK)
                    for ps, src_ in ((ps_t, tt), (ps_d2, zd2), (ps_t2, zt2)):
                        nc.tensor.matmul(
                            ps[:NCH, :],
                            lhsT=oneh[:, k, :],
                            rhs=src_[:, sl],
                            start=(k == 0),
                            stop=(k == NCH - 1),
                        )
            res_t = cpool.tile([128, CHUNK], F32, tag="res_t")
            res_t2 = cpool.tile([128, CHUNK], F32, tag="res_t2")
            res_d2 = cpool.tile([128, CHUNK], F32, tag="res_d2")
            nc.vector.tensor_copy(res_t[:NCH, :], ps_t[:NCH, :])
            nc.vector.tensor_copy(res_t2[:NCH, :], ps_t2[:NCH, :])
            nc.vector.tensor_copy(res_d2[:NCH, :], ps_d2[:NCH, :])
            nc.sync.dma_start(pt[:, :], res_t[:NCH, :])
            nc.sync.dma_start(pt2[:, :], res_t2[:NCH, :])
            nc.sync.dma_start(pd2[:, :], res_d2[:NCH, :])
    nc.compile()
    return nc


def _get_nc():
    if "nc" not in _cache:
        _cache["nc"] = _build()
    return _cache["nc"]


def _prep(y_pred, y_true, basin):
    """Counting-sort by basin, zero-pad runs to multiples of G, column layout."""
    yp = np.asarray(y_pred, dtype=np.float32).ravel()
    yt = np.asarray(y_true, dtype=np.float32).ravel()
    b = np.asarray(basin).ravel().astype(np.uint16)
    counts = np.bincount(b, minlength=N_BASINS).astype(np.int64)
    grp = -(counts // -G)  # ceil(counts/G) groups per basin
    gstart = np.zeros(N_BASINS + 1, np.int64)
    np.cumsum(grp, out=gstart[1:])
    n_used = int(gstart[-1])
    assert n_used * G <= E_TOT
    cstart = np.zeros(N_BASINS + 1, np.int64)
    np.cumsum(counts, out=cstart[1:])
    order = np.argsort(b, kind="stable")
    b_sorted = b[order]
    # padded-stream rank
    r = gstart[b_sorted] * G + (np.arange(N_TOTAL, dtype=np.int64) - cstart[b_sorted])
    # rank -> (core, tile, partition, column) -> DRAM index
    i = r % 128
    q = r // 128
    c = q // COLS
    q_core = q % COLS
    t = q_core // F_T
    col = q_core % F_T
    pos = c * E + t * (128 * F_T) + i * F_T + col
    bf = ml_dtypes.bfloat16
    yt_pad = np.zeros(E_TOT, bf)
    yp_pad = np.zeros(E_TOT, bf)
    yt_pad[pos] = yt[order]
    yp_pad[pos] = yp[order]
    return yt_pad, yp_pad, counts, grp, n_used


def _in_maps(yt_pad, yp_pad):
    return [
        {"yt": yt_pad[c * E : (c + 1) * E], "yp": yp_pad[c * E : (c + 1) * E]}
        for c in range(N_CORES)
    ]


def _finish(results, counts, grp, n_used):
    """Combine per-group device partials into the NSE mean (float64)."""

    # result [k, n] flattens to within-core group index q_core = k*CHUNK+n
    def order_groups(name):
        return np.concatenate(
            [
                np.asarray(results[c][name]).astype(np.float64).reshape(-1)
                for c in range(N_CORES)
            ]
        )[:n_used]

    gb = np.repeat(np.arange(N_BASINS), grp)
    s_t = np.bincount(gb, weights=order_groups("pt"), minlength=N_BASINS)
    s_t2 = np.bincount(gb, weights=order_groups("pt2"), minlength=N_BASINS)
    ss_res = np.bincount(gb, weights=order_groups("pd2"), minlength=N_BASINS)
    cnt = counts.astype(np.float64)
    ss_tot = s_t2 - s_t * s_t / cnt
    nse = 1.0 - ss_res / (ss_tot + EPS)
    return np.float32(nse.mean())


def kernel(y_pred, y_true, basin):
    yt_pad, yp_pad, counts, grp, n_used = _prep(y_pred, y_true, basin)
    nc = _get_nc()
    res = run_bass_kernel_spmd(nc, _in_maps(yt_pad, yp_pad), list(range(N_CORES)))
    return _finish(res.results, counts, grp, n_used)
